# revision 23
# baseline (speedup 1.0000x reference)
"""Trainium2 Bass kernel: int8-LUT-emulated 3x3 Conv2d (B=4, Cin=Cout=64, 28x28).

The LUT passed by the problem generator is the exact int8 product table
lut[i, j] = (i-128)*(j-128), so the gather-accumulate in the reference is
mathematically an integer matmul of the quantized activations and weights.
Quantized values lie in [-128, 127]; they are exactly representable in bf16,
bf16 products are exact in fp32, and the accumulated sums stay below 2^24 -
so a bf16 tensor-engine matmul with fp32 PSUM accumulation reproduces the
reference bit-exactly (up to scale-computation ulps).

Sharding (8 cores): data-parallel over batch (4) x spatial halves (2).
Each core computes out[b, :, h*14:(h+1)*14, :] = [64, 14, 28].

Per-core device schedule (all math on device; host only packs layout):
  - absmax runs on a flat replica of x ([128, 1568]) chunked across both
    HWDGE rings; the conv activations (xb2) load via the GPSIMD SWDGE queue
    so they are off the absmax critical path entirely.
  - cross-partition max folding via PE transpose (identity matmul) + free-
    dim reduce; scale broadcast back to 128 partitions with a K=1 f32r
    matmul against a constant 127-vector (single pass, unlike fp32's
    LOW/HIGH pair). The dequant scale (xmax*wmax/127^2) rides the same
    broadcast with a pre-scaled second column.
  - the weights land first and their whole scale/quantize chain finishes
    while x is still loading.
  - PE warm-up matmuls keep the tensor engine's DVFS p-state ramped so the
    real matmuls run at full clock.
  - 3x3 conv as 6 accumulating matmuls (kh=0/1 merged via a row-shifted
    SBUF copy; kh=2 as K=64 at tile_position (64,0)).
"""

import numpy as np

import concourse.bacc as bacc
import concourse.masks as masks
import concourse.mybir as mybir
import concourse.tile as tile
from concourse.bass_utils import run_bass_kernel_spmd

F32 = mybir.dt.float32
F32R = mybir.dt.float32r
BF16 = mybir.dt.bfloat16
ALU = mybir.AluOpType
AX = mybir.AxisListType
ACT_ID = mybir.ActivationFunctionType.Identity

B, C, H, W = 4, 64, 28, 28
COUT, KS, PAD = 64, 3, 1
QMAX = 127.0
MAGIC = 12582912.0  # 1.5 * 2**23: fp32 add/sub rounds to nearest-even integer

HALF = 14          # output rows per core
XB_ROWS = 16       # padded input rows held per half (14 outputs need 16 rows)
PW = W + 2 * PAD   # 30
XR_COLS = 1568     # full x, flat: 4*64*28*28 == 128 * 1568
N_CORES = 8

# x absmax chunk boundaries: sync ring carries wtx, [0:XR_A), [XR_A:XR_B),
# xb2; scalar ring carries [XR_B:XR_C) and [XR_C:XR_COLS). The sync ring
# moves roughly 2x the scalar ring's bandwidth, so it gets the bigger share.
XR_A = 470
XR_B = 760
XR_C = 1290

N_WARM1 = 4   # PE warm-ups ramping the p-state through the w chain
WARM_N = 448  # warm-up matmul free size


def _build_bass():
    nc = bacc.Bacc(None)

    xb2_d = nc.dram_tensor("xb2", [128, XB_ROWS, PW], F32, kind="ExternalInput")
    xr_d = nc.dram_tensor("xr", [128, XR_COLS], F32, kind="ExternalInput")
    wtx_d = nc.dram_tensor("wtx", [128, 6, COUT], F32, kind="ExternalInput")
    bias_d = nc.dram_tensor("biasd", [COUT, 1], F32, kind="ExternalInput")
    out_d = nc.dram_tensor("out", [COUT, HALF, W], F32, kind="ExternalOutput")

    with tile.TileContext(nc) as tc:
        with (
            tc.tile_pool(name="p", bufs=1) as pool,
            tc.tile_pool(name="ps", bufs=1, space="PSUM") as psum,
        ):
            xb2 = pool.tile([128, XB_ROWS, PW], F32, tag="xb2")
            xr = pool.tile([128, XR_COLS], F32, tag="xr")
            wtx = pool.tile([128, 6, COUT], F32, tag="wtx")
            biast = pool.tile([COUT, 1], F32, tag="bias")

            ident = pool.tile([128, 128], F32, tag="ident")
            q127 = pool.tile([1, 128], F32, tag="q127")
            wscr = pool.tile([128, COUT + WARM_N], BF16, tag="wscr")
            magict = pool.tile([128, 1], F32, tag="magict")
            nmagict = pool.tile([128, 1], F32, tag="nmagict")

            mpw = pool.tile([128, 1], F32, tag="mpw")     # w absmax partial
            px = pool.tile([128, 4], F32, tag="px")       # x absmax partials
            mpx = pool.tile([128, 1], F32, tag="mpx")     # x partials combined
            gw = pool.tile([1, 2], F32, tag="gw")         # wmax | 1/wmax
            gx = pool.tile([1, 4], F32, tag="gx")         # xmax | 1/xmax | xmax*wmax/127^3 | scratch
            tw = pool.tile([128, 6, COUT], F32, tag="tw")
            wq = pool.tile([128, 6, COUT], BF16, tag="wq")
            tx = pool.tile([128, XB_ROWS, PW], F32, tag="tx")
            xq = pool.tile([128, XB_ROWS, PW], BF16, tag="xq")
            outs = pool.tile([COUT, HALF, W], F32, tag="outs")

            rpw = pool.tile([128, 1], F32, tag="rpw")
            rpx = pool.tile([128, 1], F32, tag="rpx")
            scomb = pool.tile([128, 1], F32, tag="scomb")

            cps = psum.tile([COUT, HALF, W], F32, tag="cps")
            wps = psum.tile([COUT, WARM_N], F32, tag="wps")
            tpw = psum.tile([1, 128], F32, tag="tpw")
            tpx = psum.tile([1, 128], F32, tag="tpx")
            bc = psum.tile([128, 3], F32, tag="bc")  # rpw | rpx | scomb bcasts

            # --- constants (gpsimd; off the critical path)
            nc.gpsimd.memset(magict[:], MAGIC)
            nc.gpsimd.memset(nmagict[:], -MAGIC)
            nc.gpsimd.memset(q127[:], QMAX)
            nc.gpsimd.memset(wscr[:], 1.0)
            masks.make_identity(nc, ident[:])

            # --- input DMAs. sync ring: wtx first (it gates the early w
            # chain), then two x chunks; scalar ring: two x chunks; gpsimd
            # SWDGE: the conv activations + bias (not absmax-critical).
            nc.scalar.dma_start(xr[:, XR_B:XR_C], xr_d[:, XR_B:XR_C])
            nc.scalar.dma_start(xr[:, XR_C:XR_COLS], xr_d[:, XR_C:XR_COLS])
            nc.sync.dma_start(wtx[:], wtx_d[:])
            nc.sync.dma_start(xr[:, 0:XR_A], xr_d[:, 0:XR_A])
            nc.sync.dma_start(xr[:, XR_A:XR_B], xr_d[:, XR_A:XR_B])
            nc.sync.dma_start(xb2[:], xb2_d[:])
            nc.gpsimd.dma_start(biast[:], bias_d[:])

            # Preload the scalar engine's activation table off the critical
            # path (first ACT use otherwise pays ~1.3us mid-kernel).
            nc.scalar.activation(magict[0:1, 0:1], magict[0:1, 0:1], ACT_ID)

            # --- PE warm-up round 1 (ramps the tensor-engine p-state)
            for _ in range(N_WARM1):
                nc.tensor.matmul(
                    wps[:], wscr[:, 0:COUT], wscr[:, COUT:COUT + WARM_N],
                    start=True, stop=True)

            # --- early w chain interleaved with the x absmax partials, in
            # expected-landing order so the in-order DVE queue overlaps its
            # cross-engine waits with reduce work
            nc.vector.tensor_reduce(
                mpw[:], wtx[:], axis=AX.XY, op=ALU.max, apply_absolute_value=True)
            nc.vector.tensor_reduce(
                px[:, 2:3], xr[:, XR_B:XR_C], axis=AX.X, op=ALU.max,
                apply_absolute_value=True)
            nc.tensor.transpose(tpw[:], mpw[:], ident[:])
            nc.vector.tensor_reduce(gw[:, 0:1], tpw[:], axis=AX.X, op=ALU.max)
            nc.vector.reciprocal(gw[:, 1:2], gw[:, 0:1])
            nc.tensor.matmul(
                bc[:, 0:1], q127[:], gw[:, 1:2], start=True, stop=True)
            nc.vector.tensor_reduce(
                px[:, 0:1], xr[:, 0:XR_A], axis=AX.X, op=ALU.max,
                apply_absolute_value=True)
            nc.vector.tensor_copy(rpw[:], bc[:, 0:1])
            nc.scalar.activation(
                tw[:], wtx[:], ACT_ID, bias=magict[:], scale=rpw[:])
            nc.scalar.activation(wq[:], tw[:], ACT_ID, bias=nmagict[:])
            nc.vector.tensor_reduce(
                px[:, 3:4], xr[:, XR_C:XR_COLS], axis=AX.X, op=ALU.max,
                apply_absolute_value=True)
            nc.vector.tensor_reduce(
                px[:, 1:2], xr[:, XR_A:XR_B], axis=AX.X, op=ALU.max,
                apply_absolute_value=True)

            # --- p-state fillers gated on early x partials: bridge the PE
            # between the w broadcast and the x transpose
            nc.tensor.matmul(
                wps[:, 0:1], ident[:, 0:COUT], px[:, 2:3], start=True, stop=True)
            nc.tensor.matmul(
                wps[:, 1:2], ident[:, 0:COUT], px[:, 0:1], start=True, stop=True)

            # --- x chain. The broadcast's second column carries the dequant
            # scale: 127 * (xmax*wmax/127^3) = xmax*wmax/127^2 = sx*sw.
            nc.vector.tensor_reduce(mpx[:], px[:], axis=AX.X, op=ALU.max)
            nc.tensor.transpose(tpx[:], mpx[:], ident[:])
            nc.vector.tensor_reduce(gx[:, 0:1], tpx[:], axis=AX.X, op=ALU.max)
            nc.vector.tensor_tensor(
                gx[:, 3:4], gx[:, 0:1], gw[:, 0:1], op=ALU.mult)
            nc.vector.tensor_scalar(
                gx[:, 2:3], gx[:, 3:4], 1.0 / QMAX ** 3, None, op0=ALU.mult)
            nc.vector.reciprocal(gx[:, 1:2], gx[:, 0:1])
            nc.tensor.matmul(
                bc[:, 1:3], q127[:], gx[:, 1:3], start=True, stop=True)

            # --- x quantize: rows 0:10 on DVE, rows 10:16 on ACT (round-
            # half-even via the fp32 magic-number trick)
            RS = 10
            nc.vector.tensor_copy(rpx[:], bc[:, 1:2])
            nc.vector.tensor_scalar(
                tx[:, 0:RS, :], xb2[:, 0:RS, :], rpx[:], MAGIC,
                op0=ALU.mult, op1=ALU.add)
            nc.vector.tensor_scalar(
                xq[:, 0:RS, :], tx[:, 0:RS, :], MAGIC, None, op0=ALU.subtract)
            nc.scalar.activation(
                tx[:, RS:XB_ROWS, :], xb2[:, RS:XB_ROWS, :], ACT_ID,
                bias=magict[:], scale=rpx[:])
            nc.scalar.activation(
                xq[:, RS:XB_ROWS, :], tx[:, RS:XB_ROWS, :], ACT_ID,
                bias=nmagict[:])

            # --- PE warm-up round 3, gated on the DVE-quantized rows: keeps
            # the p-state ramped right up to the real matmuls
            for kw in range(2):
                nc.tensor.matmul(
                    wps[:, 0:280], wq[:, 5, :], xq[:, 0:RS, kw:kw + 28],
                    start=True, stop=True)

            # --- conv: 6 accumulating matmuls
            # partitions 0..63 hold padded rows r0..r0+15 (kh=0), partitions
            # 64..127 hold rows r0+1..r0+16 (kh=1 at the same row slice; kh=2
            # one slice down).
            for kw in range(3):
                nc.tensor.matmul(
                    cps[:], wq[:, kw, :], xq[:, 0:HALF, kw:kw + W],
                    start=(kw == 0), stop=False)
            for kw in range(3):
                nc.tensor.matmul(
                    cps[:], wq[64:128, 3 + kw, :], xq[64:128, 1:HALF + 1, kw:kw + W],
                    start=False, stop=(kw == 2))

            # --- dequantize + bias, store
            nc.vector.tensor_copy(scomb[:], bc[:, 2:3])
            nc.scalar.activation(
                outs[:], cps[:], ACT_ID, bias=biast[:], scale=scomb[0:COUT, :])
            nc.sync.dma_start(out_d[:], outs[:])

    nc.compile()
    return nc


_NC_CACHE = None


def _get_nc():
    global _NC_CACHE
    if _NC_CACHE is None:
        _NC_CACHE = _build_bass()
    return _NC_CACHE


def make_in_maps(x, weight, bias):
    x = np.ascontiguousarray(x, np.float32)
    weight = np.ascontiguousarray(weight, np.float32)

    # padded x with two extra zero rows so the row-shifted copy can slice
    xpad = np.zeros((B, C, H + 4, PW), np.float32)
    xpad[:, :, 1:1 + H, 1:1 + W] = x

    wt = weight.transpose(1, 2, 3, 0)  # [cin, kh, kw, cout]
    wtx = np.zeros((128, 6, COUT), np.float32)
    wtx[:64, 0:3] = wt[:, 0]
    wtx[64:, 0:3] = wt[:, 1]
    wtx[64:, 3:6] = wt[:, 2]

    biasd = np.ascontiguousarray(bias.reshape(COUT, 1), np.float32)
    xr = np.ascontiguousarray(x.reshape(128, XR_COLS))

    in_maps = []
    for core in range(N_CORES):
        b, h = divmod(core, 2)
        r0 = h * HALF
        xb_lo = xpad[b, :, r0:r0 + XB_ROWS, :]
        xb_hi = xpad[b, :, r0 + 1:r0 + 1 + XB_ROWS, :]
        xb2 = np.ascontiguousarray(np.concatenate([xb_lo, xb_hi], axis=0))

        in_maps.append({
            "xb2": xb2,
            "xr": xr,
            "wtx": wtx,
            "biasd": biasd,
        })
    return in_maps


def assemble_output(results):
    out = np.empty((B, COUT, H, W), np.float32)
    for core in range(N_CORES):
        b, h = divmod(core, 2)
        out[b, :, h * HALF:(h + 1) * HALF, :] = results[core]["out"]
    return out


def kernel(x, weight, bias, lut, **run_kwargs):
    nc = _get_nc()
    in_maps = make_in_maps(x, weight, bias)
    res = run_bass_kernel_spmd(nc, in_maps, list(range(N_CORES)), **run_kwargs)
    out = assemble_output(res.results)
    kernel.last_result = res
    return out


# revision 27
# speedup vs baseline: 1.0071x; 1.0071x over previous
"""Trainium2 Bass kernel: int8-LUT-emulated 3x3 Conv2d (B=4, Cin=Cout=64, 28x28).

The LUT passed by the problem generator is the exact int8 product table
lut[i, j] = (i-128)*(j-128), so the gather-accumulate in the reference is
mathematically an integer matmul of the quantized activations and weights.
Quantized values lie in [-128, 127]; they are exactly representable in bf16,
bf16 products are exact in fp32, and the accumulated sums stay below 2^24 -
so a bf16 tensor-engine matmul with fp32 PSUM accumulation reproduces the
reference bit-exactly (up to scale-computation ulps).

Sharding (8 cores): data-parallel over batch (4) x spatial halves (2).
Each core computes out[b, :, h*14:(h+1)*14, :] = [64, 14, 28].

Per-core device schedule (all math on device; host only packs layout):
  - absmax runs on a flat replica of x ([128, 1568]) chunked across both
    HWDGE rings; the conv activations (xb2) load via the GPSIMD SWDGE queue
    so they are off the absmax critical path entirely.
  - cross-partition max folding via PE transpose (identity matmul) + free-
    dim reduce; scale broadcast back to 128 partitions with a K=1 f32r
    matmul against a constant 127-vector (single pass, unlike fp32's
    LOW/HIGH pair). The dequant scale (xmax*wmax/127^2) rides the same
    broadcast with a pre-scaled second column.
  - the weights land first and their whole scale/quantize chain finishes
    while x is still loading.
  - PE warm-up matmuls keep the tensor engine's DVFS p-state ramped so the
    real matmuls run at full clock.
  - 3x3 conv as 6 accumulating matmuls (kh=0/1 merged via a row-shifted
    SBUF copy; kh=2 as K=64 at tile_position (64,0)).
"""

import numpy as np

import concourse.bacc as bacc
import concourse.masks as masks
import concourse.mybir as mybir
import concourse.tile as tile
from concourse.bass_utils import run_bass_kernel_spmd

F32 = mybir.dt.float32
F32R = mybir.dt.float32r
BF16 = mybir.dt.bfloat16
ALU = mybir.AluOpType
AX = mybir.AxisListType
ACT_ID = mybir.ActivationFunctionType.Identity

B, C, H, W = 4, 64, 28, 28
COUT, KS, PAD = 64, 3, 1
QMAX = 127.0
MAGIC = 12582912.0  # 1.5 * 2**23: fp32 add/sub rounds to nearest-even integer

HALF = 14          # output rows per core
XB_ROWS = 16       # padded input rows held per half (14 outputs need 16 rows)
PW = W + 2 * PAD   # 30
XR_COLS = 1568     # full x, flat: 4*64*28*28 == 128 * 1568
N_CORES = 8

# x absmax chunk boundaries: sync ring carries wtx, [0:XR_A), [XR_A:XR_B),
# xb2; scalar ring carries [XR_B:XR_C) and [XR_C:XR_COLS). The sync ring
# moves roughly 2x the scalar ring's bandwidth, so it gets the bigger share.
XR_A = 470
XR_B = 700
XR_C = 1290

N_WARM1 = 4   # PE warm-ups ramping the p-state through the w chain
WARM_N = 448  # warm-up matmul free size


def _build_bass():
    nc = bacc.Bacc(None)

    xb2_d = nc.dram_tensor("xb2", [128, XB_ROWS, PW], F32, kind="ExternalInput")
    xr_d = nc.dram_tensor("xr", [128, XR_COLS], F32, kind="ExternalInput")
    wtx_d = nc.dram_tensor("wtx", [128, 6, COUT], F32, kind="ExternalInput")
    bias_d = nc.dram_tensor("biasd", [COUT, 1], F32, kind="ExternalInput")
    out_d = nc.dram_tensor("out", [COUT, HALF, W], F32, kind="ExternalOutput")

    with tile.TileContext(nc) as tc:
        with (
            tc.tile_pool(name="p", bufs=1) as pool,
            tc.tile_pool(name="ps", bufs=1, space="PSUM") as psum,
        ):
            xb2 = pool.tile([128, XB_ROWS, PW], F32, tag="xb2")
            xr = pool.tile([128, XR_COLS], F32, tag="xr")
            wtx = pool.tile([128, 6, COUT], F32, tag="wtx")
            biast = pool.tile([COUT, 1], F32, tag="bias")

            ident = pool.tile([128, 128], F32, tag="ident")
            q127 = pool.tile([1, 128], F32, tag="q127")
            wscr = pool.tile([128, COUT + WARM_N], BF16, tag="wscr")
            magict = pool.tile([128, 1], F32, tag="magict")
            nmagict = pool.tile([128, 1], F32, tag="nmagict")

            mpw = pool.tile([128, 1], F32, tag="mpw")     # w absmax partial
            px = pool.tile([128, 4], F32, tag="px")       # x absmax partials
            mpx = pool.tile([128, 1], F32, tag="mpx")     # x partials combined
            gw = pool.tile([1, 3], F32, tag="gw")         # wmax | 1/wmax | wmax/127^3
            gx = pool.tile([1, 4], F32, tag="gx")         # xmax | 1/xmax | xmax*wmax/127^3 | scratch
            tw = pool.tile([128, 6, COUT], F32, tag="tw")
            wq = pool.tile([128, 6, COUT], BF16, tag="wq")
            tx = pool.tile([128, XB_ROWS, PW], F32, tag="tx")
            xq = pool.tile([128, XB_ROWS, PW], BF16, tag="xq")
            outs = pool.tile([COUT, HALF, W], F32, tag="outs")

            rpw = pool.tile([128, 1], F32, tag="rpw")
            rpx = pool.tile([128, 1], F32, tag="rpx")
            scomb = pool.tile([128, 1], F32, tag="scomb")

            cps = psum.tile([COUT, HALF, W], F32, tag="cps")
            wps = psum.tile([COUT, WARM_N], F32, tag="wps")
            tpw = psum.tile([1, 128], F32, tag="tpw")
            tpx = psum.tile([1, 128], F32, tag="tpx")
            bc = psum.tile([128, 3], F32, tag="bc")  # rpw | rpx | scomb bcasts

            # --- constants (gpsimd; off the critical path)
            nc.gpsimd.memset(magict[:], MAGIC)
            nc.gpsimd.memset(nmagict[:], -MAGIC)
            nc.gpsimd.memset(q127[:], QMAX)
            nc.gpsimd.memset(wscr[:], 1.0)
            masks.make_identity(nc, ident[:])

            # --- input DMAs. sync ring: wtx first (it gates the early w
            # chain), then two x chunks; scalar ring: two x chunks; gpsimd
            # SWDGE: the conv activations + bias (not absmax-critical).
            nc.scalar.dma_start(xr[:, XR_B:XR_C], xr_d[:, XR_B:XR_C])
            nc.scalar.dma_start(xr[:, XR_C:XR_COLS], xr_d[:, XR_C:XR_COLS])
            nc.sync.dma_start(wtx[:], wtx_d[:])
            nc.sync.dma_start(xr[:, 0:XR_A], xr_d[:, 0:XR_A])
            nc.sync.dma_start(xr[:, XR_A:XR_B], xr_d[:, XR_A:XR_B])
            nc.sync.dma_start(xb2[:], xb2_d[:])
            nc.gpsimd.dma_start(biast[:], bias_d[:])

            # Preload the scalar engine's activation table off the critical
            # path (first ACT use otherwise pays ~1.3us mid-kernel).
            nc.scalar.activation(magict[0:1, 0:1], magict[0:1, 0:1], ACT_ID)

            # --- PE warm-up round 1 (ramps the tensor-engine p-state)
            for _ in range(N_WARM1):
                nc.tensor.matmul(
                    wps[:], wscr[:, 0:COUT], wscr[:, COUT:COUT + WARM_N],
                    start=True, stop=True)

            # --- early w chain interleaved with the x absmax partials, in
            # expected-landing order so the in-order DVE queue overlaps its
            # cross-engine waits with reduce work
            nc.vector.tensor_reduce(
                mpw[:], wtx[:], axis=AX.XY, op=ALU.max, apply_absolute_value=True)
            nc.vector.tensor_reduce(
                px[:, 2:3], xr[:, XR_B:XR_C], axis=AX.X, op=ALU.max,
                apply_absolute_value=True)
            nc.tensor.transpose(tpw[:], mpw[:], ident[:])
            nc.vector.tensor_reduce(gw[:, 0:1], tpw[:], axis=AX.X, op=ALU.max)
            nc.vector.reciprocal(gw[:, 1:2], gw[:, 0:1])
            nc.vector.tensor_scalar(
                gw[:, 2:3], gw[:, 0:1], 1.0 / QMAX ** 3, None, op0=ALU.mult)
            nc.tensor.matmul(
                bc[:, 0:1], q127[:], gw[:, 1:2], start=True, stop=True)
            nc.vector.tensor_reduce(
                px[:, 0:1], xr[:, 0:XR_A], axis=AX.X, op=ALU.max,
                apply_absolute_value=True)
            nc.vector.tensor_copy(rpw[:], bc[:, 0:1])
            nc.scalar.activation(
                tw[:], wtx[:], ACT_ID, bias=magict[:], scale=rpw[:])
            nc.scalar.activation(wq[:], tw[:], ACT_ID, bias=nmagict[:])
            nc.vector.tensor_reduce(
                px[:, 3:4], xr[:, XR_C:XR_COLS], axis=AX.X, op=ALU.max,
                apply_absolute_value=True)
            nc.vector.tensor_reduce(
                px[:, 1:2], xr[:, XR_A:XR_B], axis=AX.X, op=ALU.max,
                apply_absolute_value=True)

            # --- p-state fillers gated on early x partials: bridge the PE
            # between the w broadcast and the x transpose
            nc.tensor.matmul(
                wps[:, 0:1], ident[:, 0:COUT], px[:, 2:3], start=True, stop=True)
            nc.tensor.matmul(
                wps[:, 1:2], ident[:, 0:COUT], px[:, 0:1], start=True, stop=True)

            # --- x chain. The broadcast's second column carries the dequant
            # scale: 127 * (xmax*wmax/127^3) = xmax*wmax/127^2 = sx*sw.
            nc.vector.tensor_reduce(mpx[:], px[:], axis=AX.X, op=ALU.max)
            nc.tensor.transpose(tpx[:], mpx[:], ident[:])
            nc.vector.tensor_reduce(gx[:, 0:1], tpx[:], axis=AX.X, op=ALU.max)
            nc.vector.tensor_tensor(
                gx[:, 2:3], gx[:, 0:1], gw[:, 2:3], op=ALU.mult)
            nc.vector.reciprocal(gx[:, 1:2], gx[:, 0:1])
            nc.tensor.matmul(
                bc[:, 1:3], q127[:], gx[:, 1:3], start=True, stop=True)

            # --- x quantize: rows 0:10 on DVE, rows 10:16 on ACT (round-
            # half-even via the fp32 magic-number trick)
            RS = 10
            nc.vector.tensor_copy(rpx[:], bc[:, 1:2])
            nc.vector.tensor_scalar(
                tx[:, 0:RS, :], xb2[:, 0:RS, :], rpx[:], MAGIC,
                op0=ALU.mult, op1=ALU.add)
            nc.vector.tensor_scalar(
                xq[:, 0:RS, :], tx[:, 0:RS, :], MAGIC, None, op0=ALU.subtract)
            nc.scalar.activation(
                tx[:, RS:XB_ROWS, :], xb2[:, RS:XB_ROWS, :], ACT_ID,
                bias=magict[:], scale=rpx[:])
            nc.scalar.activation(
                xq[:, RS:XB_ROWS, :], tx[:, RS:XB_ROWS, :], ACT_ID,
                bias=nmagict[:])

            # --- PE warm-up round 3, gated on the DVE-quantized rows: keeps
            # the p-state ramped right up to the real matmuls
            for kw in range(2):
                nc.tensor.matmul(
                    wps[:, 0:280], wq[:, 5, :], xq[:, 0:RS, kw:kw + 28],
                    start=True, stop=True)

            # --- conv: 6 accumulating matmuls
            # partitions 0..63 hold padded rows r0..r0+15 (kh=0), partitions
            # 64..127 hold rows r0+1..r0+16 (kh=1 at the same row slice; kh=2
            # one slice down).
            for kw in range(3):
                nc.tensor.matmul(
                    cps[:], wq[:, kw, :], xq[:, 0:HALF, kw:kw + W],
                    start=(kw == 0), stop=False)
            for kw in range(3):
                nc.tensor.matmul(
                    cps[:], wq[64:128, 3 + kw, :], xq[64:128, 1:HALF + 1, kw:kw + W],
                    start=False, stop=(kw == 2))

            # --- dequantize + bias, store
            nc.vector.tensor_copy(scomb[:], bc[:, 2:3])
            nc.scalar.activation(
                outs[:], cps[:], ACT_ID, bias=biast[:], scale=scomb[0:COUT, :])
            nc.sync.dma_start(out_d[:], outs[:])

    nc.compile()
    return nc


_NC_CACHE = None


def _get_nc():
    global _NC_CACHE
    if _NC_CACHE is None:
        _NC_CACHE = _build_bass()
    return _NC_CACHE


def make_in_maps(x, weight, bias):
    x = np.ascontiguousarray(x, np.float32)
    weight = np.ascontiguousarray(weight, np.float32)

    # padded x with two extra zero rows so the row-shifted copy can slice
    xpad = np.zeros((B, C, H + 4, PW), np.float32)
    xpad[:, :, 1:1 + H, 1:1 + W] = x

    wt = weight.transpose(1, 2, 3, 0)  # [cin, kh, kw, cout]
    wtx = np.zeros((128, 6, COUT), np.float32)
    wtx[:64, 0:3] = wt[:, 0]
    wtx[64:, 0:3] = wt[:, 1]
    wtx[64:, 3:6] = wt[:, 2]

    biasd = np.ascontiguousarray(bias.reshape(COUT, 1), np.float32)
    xr = np.ascontiguousarray(x.reshape(128, XR_COLS))

    in_maps = []
    for core in range(N_CORES):
        b, h = divmod(core, 2)
        r0 = h * HALF
        xb_lo = xpad[b, :, r0:r0 + XB_ROWS, :]
        xb_hi = xpad[b, :, r0 + 1:r0 + 1 + XB_ROWS, :]
        xb2 = np.ascontiguousarray(np.concatenate([xb_lo, xb_hi], axis=0))

        in_maps.append({
            "xb2": xb2,
            "xr": xr,
            "wtx": wtx,
            "biasd": biasd,
        })
    return in_maps


def assemble_output(results):
    out = np.empty((B, COUT, H, W), np.float32)
    for core in range(N_CORES):
        b, h = divmod(core, 2)
        out[b, :, h * HALF:(h + 1) * HALF, :] = results[core]["out"]
    return out


def kernel(x, weight, bias, lut, **run_kwargs):
    nc = _get_nc()
    in_maps = make_in_maps(x, weight, bias)
    res = run_bass_kernel_spmd(nc, in_maps, list(range(N_CORES)), **run_kwargs)
    out = assemble_output(res.results)
    kernel.last_result = res
    return out
